# revision 1
# baseline (speedup 1.0000x reference)
"""Trainium2 Bass kernel for nn_Alignment_vector (sparse_attention).

Reference computation per batch b (B=128, Lq=128, Ls=256, d=1024, K=256):
  q = query * matrix                                  (Lq, d)
  A = context @ q.T                                   (Ls, Lq)
  A = leaky_relu(A, 0.1); A = A / ||A||_rows(q-axis)
  attn = softmax(smooth * A.T, axis=s)                (Lq, Ls)
  wc = attn @ context; wc = wc / ||wc||_rows(d-axis)  (Lq, d)
  sim = (query - wc)^2 @ W.T + b; out = sim / ||sim||_rows

Device mapping (per core, 16 batches):
  - A computed in [s, q] layout (lhsT = contextT chunks, rhs = qT chunks);
    leaky-relu + row-l2norm + exp(smooth * a) all free-dim native on ACT/DVE.
  - softmax denominator is skipped: it cancels against the wc row-l2norm.
  - mm2 computed transposed: wcT[d, q] = sum_s context[s, d] * e[s, q], so both
    operands are in natural layouts (no on-chip transposes anywhere).
  - wc column norms via ones-matmul partition reduction; rsqrt broadcast back
    across partitions with a K=1 matmul.
  - simT[d, q] = (qT - wcT * rs)^2 feeds mm3 directly as lhsT; W bias b is
    added with one extra K=1 accumulating matmul.

Host side only reshapes/transposes/casts inputs into device-friendly layouts
and shards along the batch axis across the 8 cores.
"""

import numpy as np
import ml_dtypes

import concourse.bass as bass
import concourse.bacc as bacc
import concourse.tile as tile
from concourse import mybir
from concourse import bass_isa
from concourse.bass_utils import run_bass_kernel_spmd

B, LQ, LS, D, KS = 128, 128, 256, 1024, 256
NCORES = 8
BLOC = B // NCORES  # batches per core
DC = D // 128       # d chunks
F32 = mybir.dt.float32
AF = mybir.ActivationFunctionType
ALU = mybir.AluOpType

# matmul operand dtype (flip to False for an fp32 validation build)
MM_BF16 = True
# Prelu == leaky_relu(0.1) on HW (probe-verified); CoreSim lacks Prelu, so
# sim validation uses the equivalent max(x, 0.1x) composition instead.
USE_PRELU = True

_cache = {}


def _build(smooth: float, mm_bf16: bool, debug_taps: bool = False):
    key = (smooth, mm_bf16, USE_PRELU, debug_taps)
    if key in _cache:
        return _cache[key]

    MMDT = mybir.dt.bfloat16 if mm_bf16 else F32
    nc = bacc.Bacc("TRN2", debug=False)
    if debug_taps:
        dtal = nc.dram_tensor("dtal", (128, 2, LQ), F32, kind="ExternalOutput")
        dte = nc.dram_tensor("dte", (128, 2, LQ), MMDT, kind="ExternalOutput")
        dpsW = nc.dram_tensor("dpsW", (128, DC, LQ), F32, kind="ExternalOutput")
        dtrw = nc.dram_tensor("dtrw", (1, LQ), F32, kind="ExternalOutput")
        dtu = nc.dram_tensor("dtu", (128, DC, LQ), F32, kind="ExternalOutput")
        dpsO = nc.dram_tensor("dpsO", (128, KS), F32, kind="ExternalOutput")

    # hqm packs queryT and matrixT; hctx packs contextT and context-native
    hqm = nc.dram_tensor("hqm", (BLOC, 128, 2, DC, LQ), MMDT, kind="ExternalInput")
    hctx = nc.dram_tensor("hctx", (BLOC, 128, 2, 2048), MMDT, kind="ExternalInput")
    hw = nc.dram_tensor("hw", (128, DC, KS), MMDT, kind="ExternalInput")
    hb = nc.dram_tensor("hb", (1, KS), F32, kind="ExternalInput")
    hout = nc.dram_tensor("hout", (BLOC, LQ, KS), F32, kind="ExternalOutput")

    inv_sm2 = 1.0 / (smooth * smooth)

    with tile.TileContext(nc) as tc:
        with (
            tc.tile_pool(name="const", bufs=1) as cpool,
            tc.tile_pool(name="inp", bufs=3) as ipool,
            tc.tile_pool(name="work", bufs=3) as wpool,
            tc.tile_pool(name="ps_a", bufs=2, space="PSUM") as ps_a,
            tc.tile_pool(name="ps_w", bufs=2, space="PSUM") as ps_w,
            tc.tile_pool(name="ps_s", bufs=2, space="PSUM") as ps_s,
        ):
            tW = cpool.tile([128, DC, KS], MMDT)
            nc.sync.dma_start(out=tW, in_=hw[:, :, :])
            tb = cpool.tile([1, KS], F32)
            nc.sync.dma_start(out=tb, in_=hb[:, :])
            tones = cpool.tile([128, 1], MMDT)
            nc.vector.memset(tones, 1.0)
            tones1 = cpool.tile([1, 128], F32)
            nc.vector.memset(tones1, 1.0)

            # Stage functions over per-batch state dicts; batches are emitted
            # in stage-interleaved pairs so same-ACT-function ops cluster
            # (fewer activation-table reloads) and engines pipeline deeper.
            def s_load(v, bi):
                v["tqm"] = ipool.tile([128, 2, DC, LQ], MMDT, tag="tqm", name="tqm")
                v["tctx"] = ipool.tile([128, 2, 2048], MMDT, tag="tctx", name="tctx")
                nc.sync.dma_start(out=v["tqm"], in_=hqm[bi])
                nc.sync.dma_start(out=v["tctx"], in_=hctx[bi])
                v["tq"] = v["tqm"][:, 0]
                v["tm"] = v["tqm"][:, 1]
                v["tcT"] = v["tctx"][:, 0].rearrange("p (a s) -> p a s", a=DC)
                v["tcn"] = v["tctx"][:, 1].rearrange("p (a d) -> p a d", a=2)

            def s_qT(v, bi):
                v["tqT"] = wpool.tile([128, DC, LQ], MMDT, tag="tqT", name="tqT")
                nc.vector.tensor_mul(
                    v["tqT"].rearrange("p a q -> p (a q)"),
                    v["tq"].rearrange("p a q -> p (a q)"),
                    v["tm"].rearrange("p a q -> p (a q)"),
                )

            def s_mm1(v, bi):
                # A[s, q] = sum_d context[s, d] q[q, d]
                v["psA"] = ps_a.tile([128, 2, LQ], F32, tag="psA", name="psA")
                for i in range(2):
                    for j in range(DC):
                        nc.tensor.matmul(
                            v["psA"][:, i, :],
                            v["tcT"][:, j, 128 * i : 128 * i + 128],
                            v["tqT"][:, j, :],
                            start=(j == 0),
                            stop=(j == DC - 1),
                        )

            def s_leaky(v, bi):
                # leaky_relu(x, 0.1) = max(x, 0.1 * x) on DVE
                v["tal"] = wpool.tile([128, 2, LQ], F32, tag="tal", name="tal")
                tal01 = wpool.tile([128, 2, LQ], F32, tag="tal01")
                psA_f = v["psA"].rearrange("p a q -> p (a q)")
                tal_f = v["tal"].rearrange("p a q -> p (a q)")
                tal01_f = tal01.rearrange("p a q -> p (a q)")
                nc.vector.tensor_scalar_mul(tal01_f, psA_f, 0.1)
                nc.vector.tensor_max(tal_f, psA_f, tal01_f)
                tsqA = wpool.tile([128, 2, LQ], F32, tag="tsqA")
                nc.gpsimd.tensor_mul(
                    tsqA.rearrange("p a q -> p (a q)"), tal_f, tal_f
                )
                v["tn2A"] = wpool.tile([128, 2], F32, tag="tn2A", name="tn2A")
                for i in range(2):
                    nc.vector.reduce_sum(
                        v["tn2A"][:, i : i + 1],
                        tsqA[:, i, :],
                        axis=mybir.AxisListType.X,
                    )

            def s_anorm_sqrt(v, bi):
                # ||a_row|| / smooth, then reciprocal -> smooth / ||a_row||
                v["tnrA"] = wpool.tile([128, 2], F32, tag="tnrA", name="tnrA")
                nc.scalar.activation(v["tnrA"], v["tn2A"], AF.Sqrt, scale=inv_sm2)

            def s_anorm_recip(v, bi):
                v["trs9"] = wpool.tile([128, 2], F32, tag="trs9", name="trs9")
                nc.vector.reciprocal(v["trs9"], v["tnrA"])

            def s_exp(v, bi):
                v["te"] = wpool.tile([128, 2, LQ], MMDT, tag="te", name="te")
                for i in range(2):
                    nc.scalar.activation(
                        v["te"][:, i, :],
                        v["tal"][:, i, :],
                        AF.Exp,
                        scale=v["trs9"][:, i : i + 1],
                    )

            def s_mm2(v, bi):
                # wcT[d, q] = sum_s context[s, d] e[s, q]
                v["psW"] = ps_w.tile([128, DC, LQ], F32, tag="psW", name="psW")
                for j in range(DC):
                    for i in range(2):
                        nc.tensor.matmul(
                            v["psW"][:, j, :],
                            v["tcn"][:, i, 128 * j : 128 * j + 128],
                            v["te"][:, i, :],
                            start=(i == 0),
                            stop=(i == 1),
                        )

            def s_wsq(v, bi):
                # psN/psB/psO share one PSUM bank tile [128, 512]
                psS = ps_s.tile([128, 512], F32, tag="psS", name="psS")
                v["psN"] = psS[0:1, 0:LQ]
                v["psB"] = psS[:, LQ : 2 * LQ]
                v["psO"] = psS[:, 2 * LQ : 2 * LQ + KS]
                v["tsq"] = wpool.tile([128, DC, LQ], MMDT, tag="tsq", name="tsq")
                nc.scalar.activation(
                    v["tsq"].rearrange("p a q -> p (a q)"),
                    v["psW"].rearrange("p a q -> p (a q)"),
                    AF.Square,
                )

            def s_wones(v, bi):
                for j in range(DC):
                    nc.tensor.matmul(
                        v["psN"],
                        tones,
                        v["tsq"][:, j, :],
                        start=(j == 0),
                        stop=(j == DC - 1),
                    )

            def s_wnorm_sqrt(v, bi):
                v["tnw"] = wpool.tile([1, LQ], F32, tag="tnw", name="tnw")
                nc.scalar.activation(v["tnw"], v["psN"], AF.Sqrt)

            def s_wnorm_recip(v, bi):
                v["trw"] = wpool.tile([1, LQ], F32, tag="trw", name="trw")
                nc.vector.reciprocal(v["trw"], v["tnw"])

            def s_bcast(v, bi):
                # broadcast rs across partitions with K=1 matmul
                nc.tensor.matmul(v["psB"], tones1, v["trw"], start=True, stop=True)
                v["trwb"] = wpool.tile([128, 128], F32, tag="trwb", name="trwb")
                nc.vector.tensor_copy(v["trwb"], v["psB"])

            def s_sim(v, bi):
                # simT[d, q] = (query_T - wcT * rs)^2  (raw query!)
                trwb = v["trwb"]
                trwb_b = bass.AP(
                    tensor=trwb.tensor,
                    offset=trwb.offset,
                    ap=[list(trwb.ap[0]), [0, DC], list(trwb.ap[1])],
                )
                ttt = wpool.tile([128, DC, LQ], F32, tag="ttt")
                tu = wpool.tile([128, DC, LQ], F32, tag="tu")
                v["tu"] = tu
                v["tsim"] = wpool.tile([128, DC, LQ], MMDT, tag="tsim", name="tsim")
                nc.vector.tensor_mul(ttt, v["psW"], trwb_b)
                tu_f = tu.rearrange("p a q -> p (a q)")
                nc.vector.tensor_sub(
                    tu_f, v["tq"].rearrange("p a q -> p (a q)"),
                    ttt.rearrange("p a q -> p (a q)"),
                )
                nc.gpsimd.tensor_mul(
                    v["tsim"].rearrange("p a q -> p (a q)"), tu_f, tu_f
                )

            def s_mm3(v, bi):
                # out[q, k] = sum_d sim[q, d] W[k, d]  (+ bias b)
                for j in range(DC):
                    nc.tensor.matmul(
                        v["psO"],
                        v["tsim"][:, j, :],
                        tW[:, j, :],
                        start=(j == 0),
                        stop=False,
                    )
                nc.tensor.matmul(v["psO"], tones1, tb, start=False, stop=True)

            def s_fsq(v, bi):
                tscrF = wpool.tile([128, KS], F32, tag="tscrF")
                v["tn2f"] = wpool.tile([128, 1], F32, tag="tn2f", name="tn2f")
                nc.scalar.activation(
                    tscrF, v["psO"], AF.Square, accum_out=v["tn2f"]
                )

            def s_fnorm_sqrt(v, bi):
                v["tnf"] = wpool.tile([128, 1], F32, tag="tnf", name="tnf")
                nc.scalar.activation(v["tnf"], v["tn2f"], AF.Sqrt)

            def s_fout(v, bi):
                trf = wpool.tile([128, 1], F32, tag="trf")
                nc.vector.reciprocal(trf, v["tnf"])
                tout = wpool.tile([128, KS], F32, tag="tout")
                nc.vector.tensor_scalar_mul(tout, v["psO"], trf[:, 0:1])
                nc.sync.dma_start(out=hout[bi], in_=tout)

            def s_taps(v, bi):
                if not (debug_taps and bi == 0):
                    return
                nc.sync.dma_start(out=dtal[:, :, :], in_=v["tal"])
                nc.sync.dma_start(out=dte[:, :, :], in_=v["te"])
                dbgW = wpool.tile([128, DC, LQ], F32, tag="dbgW")
                for j in range(DC):
                    nc.scalar.copy(dbgW[:, j, :], v["psW"][:, j, :])
                nc.sync.dma_start(out=dpsW[:, :, :], in_=dbgW)
                nc.sync.dma_start(out=dtrw[:, :], in_=v["trw"])
                nc.sync.dma_start(out=dtu[:, :, :], in_=v["tu"])
                dbgO = wpool.tile([128, KS], F32, tag="dbgO")
                nc.scalar.copy(dbgO, v["psO"])
                nc.sync.dma_start(out=dpsO[:, :], in_=dbgO)

            stages = [
                s_load, s_qT, s_mm1, s_leaky, s_anorm_sqrt, s_anorm_recip,
                s_exp, s_mm2, s_wsq, s_wones, s_wnorm_sqrt, s_wnorm_recip,
                s_bcast, s_sim, s_mm3, s_fsq, s_fnorm_sqrt, s_fout, s_taps,
            ]
            for b0 in range(0, BLOC, 2):
                pair = [({}, b0), ({}, b0 + 1)]
                for stage in stages:
                    for v, bi in pair:
                        stage(v, bi)

    nc.compile()
    _cache[key] = nc
    return nc


def _prep(query, context, matrix, W, b, mm_bf16):
    npdt = ml_dtypes.bfloat16 if mm_bf16 else np.float32
    # [b, p, j, q] = query[b, q, 128j+p]
    hq = query.reshape(B, LQ, DC, 128).transpose(0, 3, 2, 1)
    hm = matrix.reshape(B, LQ, DC, 128).transpose(0, 3, 2, 1)
    hqm = np.ascontiguousarray(np.stack([hq, hm], axis=2)).astype(npdt)
    # [b, p, j, s] = context[b, s, 128j+p]
    hcT = context.reshape(B, LS, DC, 128).transpose(0, 3, 2, 1).reshape(B, 128, 2048)
    # [b, p, i, d] = context[b, 128i+p, d]
    hc = context.reshape(B, 2, 128, D).transpose(0, 2, 1, 3).reshape(B, 128, 2048)
    hctx = np.ascontiguousarray(np.stack([hcT, hc], axis=2)).astype(npdt)
    # [p, j, k] = W[k, 128j+p]
    hw = np.ascontiguousarray(W.reshape(KS, DC, 128).transpose(2, 1, 0)).astype(npdt)
    hb = np.ascontiguousarray(b.reshape(1, KS)).astype(np.float32)
    return hqm, hctx, hw, hb


def kernel(query, context, matrix, W, b, smooth, _trace=False):
    query = np.asarray(query, dtype=np.float32)
    context = np.asarray(context, dtype=np.float32)
    matrix = np.asarray(matrix, dtype=np.float32)
    W = np.asarray(W, dtype=np.float32)
    b = np.asarray(b, dtype=np.float32)

    nc = _build(float(smooth), MM_BF16)
    hqm, hctx, hw, hb = _prep(query, context, matrix, W, b, MM_BF16)

    in_maps = []
    for c in range(NCORES):
        sl = slice(c * BLOC, (c + 1) * BLOC)
        in_maps.append(
            {
                "hqm": hqm[sl],
                "hctx": hctx[sl],
                "hw": hw,
                "hb": hb,
            }
        )

    res = run_bass_kernel_spmd(
        nc, in_maps, core_ids=list(range(NCORES)), trace=_trace
    )
    out = np.concatenate([r["hout"] for r in res.results], axis=0)
    out = np.ascontiguousarray(out.astype(np.float32))
    if _trace:
        return out, res
    return out

